# revision 1
# baseline (speedup 1.0000x reference)
"""Paged GQA decode attention (B=64, HQ=32, HKV=8, D=128) on 8 TRN2 NeuronCores.

Strategy: data-parallel over requests with host-side load balancing.
 - Sort the 64 requests by context_lens descending; slot r of core c gets the
   rank-(r*8+c) request, so every core's slot-r request has a similar length.
 - Each slot is padded to the max-of-8 chunk count (chunks of 128 tokens), so
   all 8 cores execute the SAME static program (SPMD) on different data.
 - Host gathers each request's KV blocks (honoring block_tables) into per-core
   shards: K pre-transposed to [d, l] tiles (no on-chip transposes), V natural
   [l, d]. K is bf16; V fp8e4m3 (quantization errors largely cancel in the
   softmax ratio). Chunks stream in GRP-sized DMA groups that may span request
   slots (SWDGE path measured fastest at 8-core load).
 - Per chunk on device: scores_T[l,hq] = K_h^T.T @ qT (8 matmuls), then
   E = exp(scores + bias) on ScalarE where bias is 0 / -30 per token
   (masks padded/invalid tokens), then PV accumulation acc[hq,d] += E_h.T @ V_h
   (8 col-tiled matmuls into two PSUM banks) and a ones-matmul for the
   softmax denominator. Final division happens on host.
"""

import math
import os
import sys
from contextlib import ExitStack

import numpy as np
import ml_dtypes  # noqa: F401  (numpy bf16/fp8 dtypes)

for _p in ("/opt/trn_rl_repo", "/root/.axon_site/_ro/trn_rl_repo"):
    if os.path.isdir(_p) and _p not in sys.path:
        sys.path.insert(0, _p)
        break

import concourse.bass as bass  # noqa: F401
import concourse.tile as tile
from concourse import bacc, mybir
from concourse.bass_utils import run_bass_kernel_spmd

B, HQ, HKV, D, BS, MB = 64, 32, 8, 128, 16, 128
G = HQ // HKV              # 4 query heads per kv head
SCALE = 0.08838834764831845
NCORES = 8
SLOTS = B // NCORES        # 8 request slots per core
CHUNK = 128                # tokens per chunk (= SBUF partitions)
BPC = CHUNK // BS          # blocks per chunk = 8
ROW = HKV * D              # 1024 elements per token row
NEG = -30.0                # additive mask for invalid tokens
VSHIFT = -2.0              # shift valid scores so exp() fits fp8e4m3 range
GRP = 4                    # chunks per DMA group (groups may span slots)
KV_BUFS = 6                # group tiles in flight
K_ENG = "gpsimd"           # DMA issue engine for K: gpsimd|sync|scalar
V_ENG = "gpsimd"           # DMA issue engine for V
K_DT = "bf16"              # K/q dtype: "f32" | "bf16" | "fp8"
V_DT = "bf16"              # V/E dtype: "f32" | "bf16" | "fp8"

last_results = None        # stashed BassKernelResults for test.py

_prog_cache = {}


def _mdt(name):
    return {"f32": mybir.dt.float32, "bf16": mybir.dt.bfloat16,
            "fp8": mybir.dt.float8e4}[name]


def _ndt(name):
    return mybir.dt.np(_mdt(name))


def _build_program(s_counts, reps=1, dma_only=False):
    f32 = mybir.dt.float32
    kdt, vdt = _mdt(K_DT), _mdt(V_DT)
    C_total = sum(s_counts)
    NG = C_total // GRP
    nc = bacc.Bacc()

    k_d = nc.declare_dram_parameter("k", [NG, D, GRP * ROW], kdt,
                                    isOutput=False)
    v_d = nc.declare_dram_parameter("v", [NG, CHUNK, GRP * ROW], vdt,
                                    isOutput=False)
    qT_d = nc.declare_dram_parameter("qT", [D, SLOTS * HQ], kdt, isOutput=False)
    bias_d = nc.declare_dram_parameter("bias", [CHUNK, C_total], f32,
                                       isOutput=False)
    out_d = nc.declare_dram_parameter("out", [SLOTS, HKV, G, D], f32,
                                      isOutput=True)
    den_d = nc.declare_dram_parameter("den", [SLOTS, HQ], f32, isOutput=True)

    EXP = mybir.ActivationFunctionType.Exp

    with tile.TileContext(nc) as tc, ExitStack() as ctx:
        kpool = ctx.enter_context(tc.tile_pool(name="kp", bufs=KV_BUFS))
        vpool = ctx.enter_context(tc.tile_pool(name="vp", bufs=KV_BUFS))
        epool = ctx.enter_context(tc.tile_pool(name="e", bufs=3))
        const = ctx.enter_context(tc.tile_pool(name="cst", bufs=1))
        spsum = ctx.enter_context(tc.tile_pool(name="sp", bufs=2, space="PSUM"))
        apsum = ctx.enter_context(tc.tile_pool(name="ac", bufs=2, space="PSUM"))
        dpsum = ctx.enter_context(tc.tile_pool(name="dp", bufs=2, space="PSUM"))

        bias_t = const.tile([CHUNK, C_total], f32)
        nc.sync.dma_start(bias_t[:], bias_d[:])
        q_all = const.tile([D, SLOTS * HQ], kdt)
        nc.sync.dma_start(q_all[:], qT_d[:])
        # ones on ScalarE so the denominator matmul's deps stay in the single
        # ACT semaphore domain (PE matmuls support only one sync wait).
        ones = const.tile([CHUNK, 1], vdt)
        nc.scalar.activation(ones[:], bias_t[:, 0:1],
                             mybir.ActivationFunctionType.Identity,
                             bias=1.0, scale=0.0)
        # dummy matmul absorbs the q_all DMA wait so the first real matmul
        # only waits on its k/v DMA.
        dmy = spsum.tile([1, 1], f32, tag="sco")
        nc.tensor.matmul(dmy[:], q_all[0:1, 0:1], q_all[0:1, 0:1],
                         start=True, stop=True)

        def emit_body():
            cur = {}
            gc = 0
            for r in range(SLOTS):
                S_r = s_counts[r]
                qt = q_all[:, r * HQ:(r + 1) * HQ]
                acc_a = apsum.tile([CHUNK, D], f32, tag="acca")
                acc_b = apsum.tile([CHUNK, D], f32, tag="accb")
                den_p = dpsum.tile([HQ, 1], f32, tag="den")
                for j in range(S_r):
                    g, half = divmod(gc + j, GRP)
                    if half == 0 or "k" not in cur:
                        cur["k"] = kpool.tile([D, GRP * ROW], kdt,
                                              tag="kg", name="kg")
                        getattr(nc, K_ENG).dma_start(cur["k"][:], k_d[g])
                        cur["v"] = vpool.tile([CHUNK, GRP * ROW], vdt,
                                              tag="vg", name="vg")
                        getattr(nc, V_ENG).dma_start(cur["v"][:], v_d[g])
                    kt = cur["k"][:, half * ROW:(half + 1) * ROW]
                    vt = cur["v"][:, half * ROW:(half + 1) * ROW]
                    if dma_only:
                        continue

                    sco = spsum.tile([CHUNK, HQ], f32, tag="sco")
                    for h in range(HKV):
                        nc.tensor.matmul(
                            sco[:, h * G:(h + 1) * G],
                            kt[:, h * D:(h + 1) * D],
                            qt[:, h * G:(h + 1) * G],
                            start=True, stop=True,
                        )
                    et = epool.tile([CHUNK, HQ], vdt)
                    nc.scalar.activation(
                        et[:], sco[:], EXP,
                        bias=bias_t[:, gc + j:gc + j + 1], scale=1.0,
                    )
                    st, sp = (j == 0), (j == S_r - 1)
                    for h in range(HKV):
                        accp = acc_a if h < 4 else acc_b
                        jj = h % 4
                        nc.tensor.matmul(
                            accp[32 * jj:32 * jj + G, :],
                            et[:, h * G:(h + 1) * G],
                            vt[:, h * D:(h + 1) * D],
                            start=st, stop=sp,
                            tile_position=(0, 32 * jj),
                        )
                    nc.tensor.matmul(den_p[:], et[:], ones[:],
                                     start=st, stop=sp)
                out_sa = epool.tile([CHUNK, D], f32, tag="outa")
                out_sb = epool.tile([CHUNK, D], f32, tag="outb")
                den_s = epool.tile([HQ, 1], f32, tag="dens")
                if not dma_only:
                    nc.scalar.copy(out_sa[:], acc_a[:])
                    nc.scalar.copy(out_sb[:], acc_b[:])
                    nc.scalar.copy(den_s[:], den_p[:])
                else:
                    nc.vector.tensor_copy(out_sa[:], cur["k"][:, 0:D])
                    nc.vector.tensor_copy(out_sb[:], cur["v"][:, 0:D])
                    nc.vector.tensor_copy(den_s[:], bias_t[0:HQ, 0:1])
                for h in range(HKV):
                    srcp = out_sa if h < 4 else out_sb
                    jj = h % 4
                    nc.sync.dma_start(out_d[r, h], srcp[32 * jj:32 * jj + G, :])
                nc.sync.dma_start(den_d[r], den_s[:])
                gc += S_r

        if reps == 1:
            emit_body()
        else:
            with tc.For_i(0, reps, 1):
                emit_body()
    nc.compile()
    return nc


def _get_program(s_counts):
    if s_counts not in _prog_cache:
        _prog_cache[s_counts] = _build_program(s_counts)
    return _prog_cache[s_counts]


def _make_schedule(context_lens):
    L = context_lens.astype(np.int64)
    order = np.argsort(-L, kind="stable")
    s_counts = []
    for r in range(SLOTS):
        grp = order[r * NCORES:(r + 1) * NCORES]
        s_counts.append(max(1, math.ceil(int(L[grp].max()) / CHUNK)))
    rem = (-sum(s_counts)) % GRP
    s_counts[-1] += rem  # pad stream so DMA groups tile it exactly
    return order, tuple(s_counts)


def _build_in_maps(q, k_cache, v_cache, block_tables, L, order, s_counts):
    np_k, np_v = _ndt(K_DT), _ndt(V_DT)
    C_total = sum(s_counts)
    nblocks_total = k_cache.shape[0]
    kf = k_cache.reshape(nblocks_total, BS, ROW)
    vf = v_cache.reshape(nblocks_total, BS, ROW)

    in_maps = []
    core_reqs = []
    for c in range(NCORES):
        karr = np.empty((C_total, D, ROW), np_k)
        varr = np.empty((C_total, CHUNK, ROW), np_v)
        biasT = np.empty((C_total, CHUNK), np.float32)
        qT = np.empty((D, SLOTS * HQ), np_k)
        reqs = []
        gc = 0
        for r in range(SLOTS):
            b = int(order[r * NCORES + c])
            reqs.append(b)
            S_r = s_counts[r]
            blocks = np.clip(block_tables[b, :S_r * BPC].astype(np.int64),
                             0, nblocks_total - 1)
            kreq = kf[blocks].reshape(S_r, CHUNK, HKV, D)
            karr[gc:gc + S_r] = \
                kreq.transpose(0, 3, 2, 1).reshape(S_r, D, ROW)
            varr[gc:gc + S_r] = vf[blocks].reshape(S_r, CHUNK, ROW)
            tok = np.arange(S_r * CHUNK, dtype=np.int64)
            biasT[gc:gc + S_r] = np.where(tok < int(L[b]), VSHIFT, NEG) \
                .astype(np.float32).reshape(S_r, CHUNK)
            qT[:, r * HQ:(r + 1) * HQ] = (q[b] * SCALE).T
            gc += S_r
        # repack into GRP-chunk DMA groups: partition-major within a group
        kg = np.ascontiguousarray(
            karr.reshape(C_total // GRP, GRP, D, ROW).transpose(0, 2, 1, 3)
        ).reshape(C_total // GRP, D, GRP * ROW)
        vg = np.ascontiguousarray(
            varr.reshape(C_total // GRP, GRP, CHUNK, ROW).transpose(0, 2, 1, 3)
        ).reshape(C_total // GRP, CHUNK, GRP * ROW)
        in_maps.append({
            "k": kg, "v": vg, "qT": qT,
            "bias": np.ascontiguousarray(biasT.T),
        })
        core_reqs.append(reqs)
    return in_maps, core_reqs


def kernel(q, k_cache, v_cache, block_tables, context_lens):
    global last_results
    q = np.asarray(q, dtype=np.float32)
    k_cache = np.asarray(k_cache, dtype=np.float32)
    v_cache = np.asarray(v_cache, dtype=np.float32)
    block_tables = np.asarray(block_tables, dtype=np.int32)
    context_lens = np.asarray(context_lens, dtype=np.int32)

    L = context_lens.astype(np.int64)
    order, s_counts = _make_schedule(context_lens)
    nc = _get_program(s_counts)
    in_maps, core_reqs = _build_in_maps(
        q, k_cache, v_cache, block_tables, L, order, s_counts)

    res = run_bass_kernel_spmd(
        nc, in_maps, list(range(NCORES)),
        trace=bool(os.environ.get("KBASS_TRACE")),
    )
    last_results = res

    out = np.empty((B, HQ, D), np.float32)
    for c in range(NCORES):
        acc = res.results[c]["out"].reshape(SLOTS, HQ, D)
        den = np.maximum(res.results[c]["den"].reshape(SLOTS, HQ), 1e-30)
        o = acc / den[:, :, None]
        for r, b in enumerate(core_reqs[c]):
            out[b] = o[r]
    return out



# revision 6
# speedup vs baseline: 1.4519x; 1.4519x over previous
"""Paged GQA decode attention (B=64, HQ=32, HKV=8, D=128) on 8 TRN2 NeuronCores.

Strategy: data-parallel over requests with host-side load balancing.
 - Sort the 64 requests by context_lens descending; slot r of core c gets the
   rank-(r*8+c) request, so every core's slot-r request has a similar length.
 - Each slot is padded to the max-of-8 chunk count (chunks of 128 tokens), so
   all 8 cores execute the SAME static program (SPMD) on different data.
 - Host gathers each request's KV blocks (honoring block_tables) into per-core
   shards: K pre-transposed to [d, l] tiles (no on-chip transposes) in bf16;
   V in fp8e4m3-family (fp8e3 = e3m4) with a per-head ones/mask column
   appended, and invalid-token rows zeroed.  The mask column makes the PV
   matmul emit the softmax denominator for free, and the zeroed V rows mask
   padded/invalid tokens without any score bias.
 - Chunks stream in GRP-sized DMA groups that may span request slots.
 - Per chunk on device: 8 score matmuls (K_h^T stationary, q streaming) into
   a group-wide PSUM tile; one ScalarE exp per GROUP (constant -2 shift keeps
   E in bf16 range); 8 col-tiled PV matmuls per chunk accumulate
   acc[head-strip, D+1] per slot (last column = denominator).  Slot epilogue:
   one DVE PSUM->SBUF copy + one DMA.  Final division happens on host.
"""

import math
import os
import sys
from contextlib import ExitStack

import numpy as np
import ml_dtypes  # noqa: F401  (numpy bf16/fp8 dtypes)

for _p in ("/opt/trn_rl_repo", "/root/.axon_site/_ro/trn_rl_repo"):
    if os.path.isdir(_p) and _p not in sys.path:
        sys.path.insert(0, _p)
        break

import concourse.bass as bass  # noqa: F401
import concourse.tile as tile
from concourse import bacc, mybir
from concourse.bass_utils import run_bass_kernel_spmd

B, HQ, HKV, D, BS, MB = 64, 32, 8, 128, 16, 128
G = HQ // HKV              # 4 query heads per kv head
SCALE = 0.08838834764831845
NCORES = 8
SLOTS = B // NCORES        # 8 request slots per core
CHUNK = 128                # tokens per chunk (= SBUF partitions)
BPC = CHUNK // BS          # blocks per chunk = 8
ROW = HKV * D              # 1024 K elements per token row
DV = D + 1                 # V row per head incl. mask column
ROWV = HKV * DV            # 1032 V elements per token row
GRP = 4                    # chunks per DMA group (groups may span slots)
KV_BUFS = 8                # group tiles in flight
K_ENG = "gpsimd"           # DMA issue engine for K: gpsimd|sync|scalar
V_ENG = "gpsimd"           # DMA issue engine for V
K_DT = "bf16"              # K/q dtype: "f32" | "bf16" | "fp8"
V_DT = "fp8"               # V dtype: "f32" | "bf16" | "fp8"

last_results = None        # stashed BassKernelResults for test.py

_prog_cache = {}


def _mdt(name):
    return {"f32": mybir.dt.float32, "bf16": mybir.dt.bfloat16,
            "fp8": mybir.dt.float8e3}[name]


def _ndt(name):
    return mybir.dt.np(_mdt(name))


def _build_program(s_counts, dma_only=False):
    f32 = mybir.dt.float32
    kdt, vdt = _mdt(K_DT), _mdt(V_DT)
    edt = mybir.dt.bfloat16  # E (softmax numerator) dtype
    C_total = sum(s_counts)
    NG = C_total // GRP
    nc = bacc.Bacc()

    k_d = nc.declare_dram_parameter("k", [NG, D, GRP * ROW], kdt,
                                    isOutput=False)
    v_d = nc.declare_dram_parameter("v", [NG, CHUNK, GRP * ROWV], vdt,
                                    isOutput=False)
    qT_d = nc.declare_dram_parameter("qT", [D, SLOTS * HQ], kdt, isOutput=False)
    out_d = nc.declare_dram_parameter("out", [SLOTS, CHUNK, 2 * DV], f32,
                                      isOutput=True)

    EXP = mybir.ActivationFunctionType.Exp

    # chunk -> owning slot, first/last flags
    slot_of, first_of, last_of = [], [], []
    for r, S_r in enumerate(s_counts):
        for j in range(S_r):
            slot_of.append(r)
            first_of.append(j == 0)
            last_of.append(j == S_r - 1)

    with tile.TileContext(nc) as tc, ExitStack() as ctx:
        kpool = ctx.enter_context(tc.tile_pool(name="kp", bufs=KV_BUFS))
        vpool = ctx.enter_context(tc.tile_pool(name="vp", bufs=KV_BUFS))
        epool = ctx.enter_context(tc.tile_pool(name="e", bufs=3))
        opool = ctx.enter_context(tc.tile_pool(name="o", bufs=2))
        const = ctx.enter_context(tc.tile_pool(name="cst", bufs=1))
        spsum = ctx.enter_context(tc.tile_pool(name="sp", bufs=2, space="PSUM"))
        apsum = ctx.enter_context(tc.tile_pool(name="ac", bufs=2, space="PSUM"))

        q_all = const.tile([D, SLOTS * HQ], kdt)
        nc.sync.dma_start(q_all[:], qT_d[:])
        # dummy matmul absorbs the q_all DMA wait so the first real matmul
        # only waits on its k DMA (PE matmuls support only one sync wait).
        dmy = spsum.tile([1, 1], f32, tag="sco")
        nc.tensor.matmul(dmy[:], q_all[0:1, 0:1], q_all[0:1, 0:1],
                         start=True, stop=True)

        acc = None
        for g in range(NG):
            kg = kpool.tile([D, GRP * ROW], kdt, tag="kg", name="kg")
            getattr(nc, K_ENG).dma_start(kg[:], k_d[g])
            vg = vpool.tile([CHUNK, GRP * ROWV], vdt, tag="vg", name="vg")
            getattr(nc, V_ENG).dma_start(vg[:], v_d[g])
            if dma_only:
                ot = opool.tile([CHUNK, 2 * DV], f32, tag="out")
                nc.vector.tensor_copy(ot[:, 0:DV], kg[:, 0:DV])
                nc.vector.tensor_copy(ot[:, DV:2 * DV], vg[:, 0:DV])
                if g % 8 == 7:
                    nc.sync.dma_start(out_d[slot_of[g * GRP]], ot[:])
                continue

            sco = spsum.tile([CHUNK, GRP * HQ], f32, tag="sco")
            for half in range(GRP):
                c = g * GRP + half
                r = slot_of[c]
                kt = kg[:, half * ROW:(half + 1) * ROW]
                for h in range(HKV):
                    nc.tensor.matmul(
                        sco[:, half * HQ + h * G:half * HQ + (h + 1) * G],
                        kt[:, h * D:(h + 1) * D],
                        q_all[:, r * HQ + h * G:r * HQ + (h + 1) * G],
                        start=True, stop=True,
                    )
            et = epool.tile([CHUNK, GRP * HQ], edt)
            nc.scalar.activation(et[:], sco[:], EXP, bias=0.0, scale=1.0)

            for half in range(GRP):
                c = g * GRP + half
                r = slot_of[c]
                if first_of[c]:
                    # separate banks per head-half: a start=True clears
                    # has_written for the whole bank on the written
                    # partitions, so the two halves must not share a bank.
                    acc_a = apsum.tile([CHUNK, DV], f32, tag="acca",
                                       name="acc_a")
                    acc_b = apsum.tile([CHUNK, DV], f32, tag="accb",
                                       name="acc_b")
                    acc = (acc_a, acc_b)
                st, sp = first_of[c], last_of[c]
                for h in range(HKV):
                    jj = h % G
                    nc.tensor.matmul(
                        acc[h // G][32 * jj:32 * jj + G, :],
                        et[:, half * HQ + h * G:half * HQ + (h + 1) * G],
                        vg[:, half * ROWV + h * DV:half * ROWV + (h + 1) * DV],
                        start=st, stop=sp,
                        tile_position=(0, 32 * jj),
                    )
                if last_of[c]:
                    ot = opool.tile([CHUNK, 2 * DV], f32, tag="out")
                    nc.vector.tensor_copy(ot[:, 0:DV], acc[0][:])
                    nc.vector.tensor_copy(ot[:, DV:2 * DV], acc[1][:])
                    nc.sync.dma_start(out_d[r], ot[:])
    nc.compile()
    return nc


def _get_program(s_counts):
    if s_counts not in _prog_cache:
        _prog_cache[s_counts] = _build_program(s_counts)
    return _prog_cache[s_counts]


def _make_schedule(context_lens):
    L = context_lens.astype(np.int64)
    order = np.argsort(-L, kind="stable")
    s_counts = []
    for r in range(SLOTS):
        grp = order[r * NCORES:(r + 1) * NCORES]
        s_counts.append(max(1, math.ceil(int(L[grp].max()) / CHUNK)))
    rem = (-sum(s_counts)) % GRP
    s_counts[-1] += rem  # pad stream so DMA groups tile it exactly
    return order, tuple(s_counts)


def _build_in_maps(q, k_cache, v_cache, block_tables, L, order, s_counts):
    np_k, np_v = _ndt(K_DT), _ndt(V_DT)
    C_total = sum(s_counts)
    nblocks_total = k_cache.shape[0]
    kf = k_cache.reshape(nblocks_total, BS, ROW)
    vf = v_cache.reshape(nblocks_total, BS, HKV, D)

    in_maps = []
    core_reqs = []
    for c in range(NCORES):
        karr = np.empty((C_total, D, ROW), np_k)
        varr = np.zeros((C_total, CHUNK, HKV, DV), np.float32)
        qT = np.empty((D, SLOTS * HQ), np_k)
        reqs = []
        gc = 0
        for r in range(SLOTS):
            b = int(order[r * NCORES + c])
            reqs.append(b)
            S_r = s_counts[r]
            blocks = np.clip(block_tables[b, :S_r * BPC].astype(np.int64),
                             0, nblocks_total - 1)
            kreq = kf[blocks].reshape(S_r, CHUNK, HKV, D)
            karr[gc:gc + S_r] = \
                kreq.transpose(0, 3, 2, 1).reshape(S_r, D, ROW)
            nv = min(int(L[b]), S_r * CHUNK)
            vreq = vf[blocks].reshape(S_r * CHUNK, HKV, D)
            vslab = varr[gc:gc + S_r].reshape(S_r * CHUNK, HKV, DV)
            vslab[:nv, :, :D] = vreq[:nv]
            vslab[:nv, :, D] = 1.0
            qT[:, r * HQ:(r + 1) * HQ] = (q[b] * SCALE).T
            gc += S_r
        # repack into GRP-chunk DMA groups: partition-major within a group
        kg = np.ascontiguousarray(
            karr.reshape(C_total // GRP, GRP, D, ROW).transpose(0, 2, 1, 3)
        ).reshape(C_total // GRP, D, GRP * ROW)
        vg = np.ascontiguousarray(
            varr.reshape(C_total // GRP, GRP, CHUNK, ROWV)
            .transpose(0, 2, 1, 3)
        ).reshape(C_total // GRP, CHUNK, GRP * ROWV).astype(np_v)
        in_maps.append({"k": kg, "v": vg, "qT": qT})
        core_reqs.append(reqs)
    return in_maps, core_reqs


def kernel(q, k_cache, v_cache, block_tables, context_lens):
    global last_results
    q = np.asarray(q, dtype=np.float32)
    k_cache = np.asarray(k_cache, dtype=np.float32)
    v_cache = np.asarray(v_cache, dtype=np.float32)
    block_tables = np.asarray(block_tables, dtype=np.int32)
    context_lens = np.asarray(context_lens, dtype=np.int32)

    L = context_lens.astype(np.int64)
    order, s_counts = _make_schedule(context_lens)
    nc = _get_program(s_counts)
    in_maps, core_reqs = _build_in_maps(
        q, k_cache, v_cache, block_tables, L, order, s_counts)

    res = run_bass_kernel_spmd(
        nc, in_maps, list(range(NCORES)),
        trace=bool(os.environ.get("KBASS_TRACE")),
    )
    last_results = res

    out = np.empty((B, HQ, D), np.float32)
    for c in range(NCORES):
        raw = res.results[c]["out"].reshape(SLOTS, CHUNK, 2 * DV)
        for r, b in enumerate(core_reqs[c]):
            for h in range(HKV):
                jj = h % G
                blk = 0 if h < G else DV
                num = raw[r, 32 * jj:32 * jj + G, blk:blk + D]
                den = raw[r, 32 * jj:32 * jj + G, blk + D]
                out[b, h * G:(h + 1) * G] = \
                    num / np.maximum(den, 1e-30)[:, None]
    return out
